# revision 8
# baseline (speedup 1.0000x reference)
"""DRL4TSP greedy pointer-network decode on 8 Trainium2 NeuronCores.

Pure batch data parallelism: 2048 rows -> 8 cores x 256 rows. All weights
replicated. Per core, the S=128 sequential decode steps run on-chip:

  layouts (per core, BC=256 rows split into 2 chunks of 128 "cb"):
    base      SBUF [h=128, b=256, s=128] fp32   (loop-invariant attention term)
    sh_table  DRAM [b*S=32768, h=128]    fp32   (static_hidden rows for gather)
    state     SBUF [h=128, b=128] per cb        (h_t, c_t, dec_t, q_t)

  per step: LSTM gates (PE fp32) -> sigmoid-via-tanh / tanh (ACT LUT) ->
  q proj (PE) -> u = base + q broadcast (DVE) -> tanh(u) (ACT) ->
  attns = w . tanh(u) as per-b self-loading fp32 matmuls (PE, psum [s,b]) ->
  transpose -> softmax over s>=1 with the reference's +10000 fp32
  quantization reproduced -> argmax with first-index tie-break (DVE) ->
  indirect-DMA gather of the next decoder input.

Numerics: ACT's tanh/exp/log LUTs are bit-identical to XLA-CPU's, fp32 PE
matmuls are fp32-faithful, so attns track the CPU-jax reference to ~1e-7,
which the +10000 quantization (grid ~1e-3) absorbs.
"""
import os
import time
from contextlib import ExitStack

import numpy as np

import concourse.bass as bass
import concourse.tile as tile
from concourse import bacc, mybir
from concourse.bass_utils import run_bass_kernel_spmd
from concourse.masks import make_identity

dt = mybir.dt
F32 = dt.float32
I32 = dt.int32
AF = mybir.ActivationFunctionType
ALU = mybir.AluOpType
AX = mybir.AxisListType

NCORES = 8
B = 2048
BC = B // NCORES          # 256 rows per core
S = 128                   # sequence length == number of decode steps
H = 128                   # hidden
NCB = BC // 128           # 2 b-chunks of 128 per core
CHUNK_B = 16              # b's per u/tanh chunk (chunk = [128, 16*128] cols)

WNAMES = [
    ("W_s", (H, 2)), ("b_s", (H,)), ("W_ld", (H, 2)), ("b_ld", (H,)),
    ("W_d", (H, 2)), ("b_d", (H,)),
    ("W_ih", (4 * H, H)), ("b_ih", (4 * H,)), ("W_hh", (4 * H, H)), ("b_hh", (4 * H,)),
    ("W_pd", (H, H)), ("b_pd", (H,)), ("W_pld", (H, H)), ("b_pld", (H,)),
    ("W_pq", (H, H)), ("b_pq", (H,)), ("W_pr", (H, H)), ("b_pr", (H,)),
    ("attn_W", (1, H)),
]


def build_body(tc, ins, outs, n_steps=S):
    """Emit the whole per-core program. ins/outs: dicts name -> DRAM AP."""
    nc = tc.nc
    with ExitStack() as ctx:
        const = ctx.enter_context(tc.tile_pool(name="const", bufs=1))
        basep = ctx.enter_context(tc.tile_pool(name="basep", bufs=1))
        accp = ctx.enter_context(tc.tile_pool(name="accp", bufs=1))
        dramp = ctx.enter_context(tc.tile_pool(name="dramp", bufs=1, space="DRAM"))
        state = ctx.enter_context(tc.tile_pool(name="state", bufs=2))
        work = ctx.enter_context(tc.tile_pool(name="work", bufs=2))
        small = ctx.enter_context(tc.tile_pool(name="small", bufs=2))
        psg = ctx.enter_context(tc.tile_pool(name="psg", bufs=2, space="PSUM"))
        psq = ctx.enter_context(tc.tile_pool(name="psq", bufs=2, space="PSUM"))
        psattn = ctx.enter_context(tc.tile_pool(name="psattn", bufs=2, space="PSUM"))
        pst = ctx.enter_context(tc.tile_pool(name="pst", bufs=2, space="PSUM"))

        ident = const.tile([128, 128], F32, tag="ident")
        make_identity(nc, ident[:])

        def transpose128(src_ap, dst_ap, engine=None):
            p = pst.tile([128, 128], F32, tag="pst")
            nc.tensor.transpose(p[:], src_ap, ident[:])
            if engine is nc.scalar:
                nc.scalar.copy(dst_ap, p[:])
            else:
                nc.vector.tensor_copy(dst_ap, p[:])

        # ---------------- Phase 0: weights prep ----------------
        def load_t(name, shape, tag):
            t = const.tile(list(shape), F32, tag=tag)
            nc.sync.dma_start(t[:], ins[name][:])
            return t

        # [512,128] -> 4 transposed blocks [128(k),128(gate_h)]
        def blocks_T(name, tag):
            blks = []
            for g in range(4):
                raw = small.tile([128, 128], F32, tag="rawblk")
                nc.sync.dma_start(raw[:], ins[name][g * 128:(g + 1) * 128, :])
                tt = const.tile([128, 128], F32, tag=f"{tag}{g}")
                transpose128(raw[:], tt[:])
                blks.append(tt)
            return blks

        W_ihT = blocks_T("W_ih", "wihT")
        W_hhT = blocks_T("W_hh", "whhT")

        def square_T(name, tag):
            raw = small.tile([128, 128], F32, tag="rawblk")
            nc.sync.dma_start(raw[:], ins[name][:])
            tt = const.tile([128, 128], F32, tag=tag)
            transpose128(raw[:], tt[:])
            return tt

        W_pqT = square_T("W_pq", "wpqT")
        W_prT = square_T("W_pr", "wprT")
        W_pldT = square_T("W_pld", "wpldT")
        W_pdT = square_T("W_pd", "wpdT")

        # bias columns [128,1] via partition-scatter DMA
        def col_of(vec_ap, tag, n=128):
            c = const.tile([n, 1], F32, tag=tag)
            nc.sync.dma_start(c[:], vec_ap.unsqueeze(1))
            return c

        b_s_c = col_of(ins["b_s"][:], "bsc")
        b_ld_c = col_of(ins["b_ld"][:], "bldc")
        b_d_c = col_of(ins["b_d"][:], "bdc")
        b_pq_c = col_of(ins["b_pq"][:], "bpqc")
        wcol = col_of(ins["attn_W"][0, :], "wcol")

        # gate biases: bg = b_ih + b_hh as [128, 4]; bgh = 0.5*bg
        bihc = const.tile([128, 4], F32, tag="bihc")
        bhhc = const.tile([128, 4], F32, tag="bhhc")
        nc.sync.dma_start(bihc[:], ins["b_ih"][:].rearrange("(g p) -> p g", p=128))
        nc.sync.dma_start(bhhc[:], ins["b_hh"][:].rearrange("(g p) -> p g", p=128))
        bg = const.tile([128, 4], F32, tag="bg")
        nc.vector.tensor_add(bg[:], bihc[:], bhhc[:])
        bgh = const.tile([128, 4], F32, tag="bgh")
        nc.vector.tensor_scalar_mul(bgh[:], bg[:], 0.5)

        # W_s [128,2]; effective 1-col sums of W_ld / W_d (einsum broadcast)
        W_s_sb = load_t("W_s", (128, 2), "wssb")
        W_ld_sb = load_t("W_ld", (128, 2), "wldsb")
        W_d_sb = load_t("W_d", (128, 2), "wdsb")
        wldsum = const.tile([128, 1], F32, tag="wldsum")
        nc.vector.tensor_reduce(wldsum[:], W_ld_sb[:], axis=AX.X, op=ALU.add)
        wdsum = const.tile([128, 1], F32, tag="wdsum")
        nc.vector.tensor_reduce(wdsum[:], W_d_sb[:], axis=AX.X, op=ALU.add)

        # feature row order: 0=ONES, 1=X0, 2=X1, 3=LD, 4=DD.
        # Engine ops must start at partition 0, so rows are staged at
        # partition 0 and DMA'd into place.
        lhsT_base = const.tile([5, 128], F32, tag="lhsTbase")
        lhsT_sh = const.tile([5, 128], F32, tag="lhsTsh")

        def stage_to(dst_row_ap, psum_ap, nrows):
            stg = small.tile([nrows, 128], F32, tag="stgrow", name="stgrow")
            nc.vector.tensor_copy(stg[:], psum_ap)
            nc.sync.dma_start(dst_row_ap, stg[:])

        # rows 1:3 of lhsT_base = A_sT = W_s^T @ W_pr^T
        p = pst.tile([128, 128], F32, tag="pst")
        nc.tensor.matmul(p[0:2, :], W_s_sb[:], W_prT[:], start=True, stop=True)
        stage_to(lhsT_base[1:3, :], p[0:2, :], 2)
        # row 3 = A_ldT (effective 1-col), row 4 = A_dT
        p = pst.tile([128, 128], F32, tag="pst")
        nc.tensor.matmul(p[0:1, :], wldsum[:], W_pldT[:], start=True, stop=True)
        stage_to(lhsT_base[3:4, :], p[0:1, :], 1)
        p = pst.tile([128, 128], F32, tag="pst")
        nc.tensor.matmul(p[0:1, :], wdsum[:], W_pdT[:], start=True, stop=True)
        stage_to(lhsT_base[4:5, :], p[0:1, :], 1)
        # row 0 = btot = W_pr@b_s + W_pld@b_ld + W_pd@b_d + b_pr + b_pld + b_pd
        p = pst.tile([128, 128], F32, tag="pst")
        nc.tensor.matmul(p[0:1, :], b_s_c[:], W_prT[:], start=True, stop=False)
        nc.tensor.matmul(p[0:1, :], b_ld_c[:], W_pldT[:], start=False, stop=False)
        nc.tensor.matmul(p[0:1, :], b_d_c[:], W_pdT[:], start=False, stop=True)
        btot = small.tile([1, 128], F32, tag="btot")
        nc.vector.tensor_copy(btot[:], p[0:1, :])
        for nm in ("b_pr", "b_pld", "b_pd"):
            brow = small.tile([1, 128], F32, tag="brow")
            nc.sync.dma_start(brow[:], ins[nm][:].unsqueeze(0))
            nc.vector.tensor_add(btot[:], btot[:], brow[:])
        nc.sync.dma_start(lhsT_base[0:1, :], btot[:])

        # lhsT_sh rows: 0 = b_s, 1:3 = W_sT, 3:5 = zeros
        bsrow = small.tile([1, 128], F32, tag="brow")
        nc.sync.dma_start(bsrow[:], ins["b_s"][:].unsqueeze(0))
        nc.sync.dma_start(lhsT_sh[0:1, :], bsrow[:])
        p = pst.tile([128, 128], F32, tag="pst")
        nc.tensor.matmul(p[0:2, :], W_s_sb[:], ident[:], start=True, stop=True)
        stage_to(lhsT_sh[1:3, :], p[0:2, :], 2)
        zrow = small.tile([1, 128], F32, tag="zrow")
        nc.vector.memset(zrow[:], 0.0)
        nc.sync.dma_start(lhsT_sh[3:4, :], zrow[:])
        nc.sync.dma_start(lhsT_sh[4:5, :], zrow[:])

        # iotas
        revio_i = const.tile([128, S - 1], I32, tag="revioi")
        nc.gpsimd.iota(revio_i[:], pattern=[[-1, S - 1]], base=S - 2, channel_multiplier=0)
        revio = const.tile([128, S - 1], F32, tag="revio")
        nc.vector.tensor_copy(revio[:], revio_i[:])
        biota = []
        for cb in range(NCB):
            bi = const.tile([128, 1], I32, tag=f"biotai{cb}")
            nc.gpsimd.iota(bi[:], pattern=[[0, 1]], base=cb * 128 * S, channel_multiplier=S)
            bf = const.tile([128, 1], F32, tag=f"biotaf{cb}")
            nc.vector.tensor_copy(bf[:], bi[:])
            biota.append(bf)

        # ---------------- Phase 1: base + sh_table ----------------
        base_t = basep.tile([128, BC, S], F32, tag="base")
        sh_table = dramp.tile([BC * S, H], F32, tag="shtab")

        # LD = load - demand, staged via DRAM so phase-1 chunks can slice rows
        LDfull = const.tile([128, NCB, S], F32, tag="ldfull")
        ld_dram = dramp.tile([BC, S], F32, tag="lddram", name="lddram")
        for cb in range(NCB):
            dyn = work.tile([128, 2, S], F32, tag="u")
            nc.sync.dma_start(dyn[:], ins["dynamic"][cb * 128:(cb + 1) * 128, :, :])
            nc.vector.tensor_sub(LDfull[:, cb, :], dyn[:, 0, :], dyn[:, 1, :])
            nc.sync.dma_start(ld_dram[cb * 128:(cb + 1) * 128, :], LDfull[:, cb, :])

        st_r = ins["static"].rearrange("b c s -> c b s")
        dy_r = ins["dynamic"].rearrange("b c s -> c b s")
        NB_CH = 16                      # b's per phase-1 chunk (2048 cols)
        for ch in range(BC // NB_CH):
            b0 = ch * NB_CH
            feat = work.tile([5, NB_CH, S], F32, tag="u")
            nc.vector.memset(feat[0:1, :, :], 1.0)
            nc.sync.dma_start(feat[1:2, :, :], st_r[0:1, b0:b0 + NB_CH, :])
            nc.sync.dma_start(feat[2:3, :, :], st_r[1:2, b0:b0 + NB_CH, :])
            nc.sync.dma_start(feat[3:4, :, :], ld_dram[b0:b0 + NB_CH, :].unsqueeze(0))
            nc.sync.dma_start(feat[4:5, :, :], dy_r[1:2, b0:b0 + NB_CH, :])
            featf = feat[:].rearrange("k b s -> k (b s)")
            for sub in range(NB_CH * S // 512):
                cols = featf[:, sub * 512:(sub + 1) * 512]
                pb = psq.tile([128, 512], F32, tag="q")
                nc.tensor.matmul(pb[:], lhsT_base[:], cols, start=True, stop=True)
                nc.vector.tensor_copy(
                    base_t[:].rearrange("h b s -> h (b s)")[:, ch * NB_CH * S + sub * 512:][:, :512],
                    pb[:])
                psh = psg.tile([128, 512], F32, tag="gates")
                nc.tensor.matmul(psh[:], lhsT_sh[:], cols, start=True, stop=True)
                n0 = ch * NB_CH * S + sub * 512
                for blk in range(4):
                    stg = small.tile([128, 128], F32, tag="stg")
                    nc.vector.tensor_copy(stg[:], psh[:, blk * 128:(blk + 1) * 128])
                    stT = small.tile([128, 128], F32, tag="stT")
                    transpose128(stg[:], stT[:], engine=nc.scalar)
                    nc.sync.dma_start(
                        sh_table[n0 + blk * 128:n0 + (blk + 1) * 128, :], stT[:])

        # ---------------- Phase 2: decode loop ----------------
        # initial state
        h_cb, c_cb, dec_cb = [], [], []
        for cb in range(NCB):
            h0 = state.tile([128, 128], F32, tag=f"h{cb}")
            nc.vector.memset(h0[:], 0.0)
            c0 = state.tile([128, 128], F32, tag=f"c{cb}")
            nc.vector.memset(c0[:], 0.0)
            dg = small.tile([128, 128], F32, tag="decg")
            nc.sync.dma_start(
                dg[:],
                sh_table[:].rearrange("(b s) h -> b s h", s=S)[cb * 128:(cb + 1) * 128, 0, :])
            d0 = state.tile([128, 128], F32, tag=f"dec{cb}")
            transpose128(dg[:], d0[:])
            h_cb.append(h0); c_cb.append(c0); dec_cb.append(d0)

        Zbuf, Pbuf = [], []
        for cb in range(NCB):
            Zbuf.append(accp.tile([128, S], F32, tag=f"zbuf{cb}", name=f"zbuf{cb}"))
            Pbuf.append(accp.tile([128, S], I32, tag=f"pbuf{cb}", name=f"pbuf{cb}"))

        NCH = 128 // CHUNK_B            # u/tanh chunks per cb
        for t in range(n_steps):
            for cb in range(NCB):
                # --- LSTM ---
                pg = psg.tile([128, 4, 128], F32, tag="gates")
                for g in range(4):
                    nc.tensor.matmul(pg[:, g, :], W_ihT[g][:], dec_cb[cb][:],
                                     start=True, stop=False)
                    nc.tensor.matmul(pg[:, g, :], W_hhT[g][:], h_cb[cb][:],
                                     start=False, stop=True)
                gs = []
                for g in (0, 1, 3):     # i, f, o: sigmoid via tanh
                    th = small.tile([128, 128], F32, tag=f"gth{g}")
                    nc.scalar.activation(th[:], pg[:, g, :], AF.Tanh,
                                         bias=bgh[:, g:g + 1], scale=0.5)
                    sg = small.tile([128, 128], F32, tag=f"gsg{g}")
                    nc.vector.tensor_scalar(sg[:], th[:], 0.5, 0.5,
                                            op0=ALU.mult, op1=ALU.add)
                    gs.append(sg)
                i_s, f_s, o_s = gs
                g_t = small.tile([128, 128], F32, tag="gcell")
                nc.scalar.activation(g_t[:], pg[:, 2, :], AF.Tanh,
                                     bias=bg[:, 2:3], scale=1.0)
                t1 = small.tile([128, 128], F32, tag="t1")
                nc.vector.tensor_mul(t1[:], f_s[:], c_cb[cb][:])
                t2 = small.tile([128, 128], F32, tag="t2")
                nc.vector.tensor_mul(t2[:], i_s[:], g_t[:])
                c_new = state.tile([128, 128], F32, tag=f"c{cb}")
                nc.vector.tensor_add(c_new[:], t1[:], t2[:])
                ct = small.tile([128, 128], F32, tag="ct")
                nc.scalar.activation(ct[:], c_new[:], AF.Tanh)
                h_new = state.tile([128, 128], F32, tag=f"h{cb}")
                nc.vector.tensor_mul(h_new[:], o_s[:], ct[:])
                c_cb[cb] = c_new; h_cb[cb] = h_new

                # --- q ---
                pq = psq.tile([128, 128], F32, tag="q")
                nc.tensor.matmul(pq[:], W_pqT[:], h_new[:], start=True, stop=True)
                qS = state.tile([128, 128], F32, tag=f"q{cb}")
                nc.scalar.activation(qS[:], pq[:], AF.Identity, bias=b_pq_c[:])

                # --- u = base + q, tanh, attn matmuls ---
                pa = psattn.tile([128, 128], F32, tag="attn")
                for ch in range(NCH):
                    bl = cb * 128 + ch * CHUNK_B     # global b of chunk start
                    u = work.tile([128, CHUNK_B, S], F32, tag="u")
                    nc.vector.tensor_tensor(
                        u[:], base_t[:, bl:bl + CHUNK_B, :],
                        qS[:, ch * CHUNK_B:(ch + 1) * CHUNK_B].unsqueeze(2)
                          .broadcast_to([128, CHUNK_B, S]),
                        op=ALU.add)
                    tt = work.tile([128, CHUNK_B, S], F32, tag="tt")
                    nc.scalar.activation(tt[:], u[:], AF.Tanh)
                    for j in range(CHUNK_B):
                        jb = ch * CHUNK_B + j
                        nc.tensor.matmul(pa[:, jb:jb + 1], tt[:, j, :], wcol[:],
                                         start=True, stop=True)

                # --- evac + transpose to [b, s] ---
                aT = small.tile([128, 128], F32, tag="aT")
                nc.scalar.copy(aT[:], pa[:])
                att = small.tile([128, 128], F32, tag="att")
                transpose128(aT[:], att[:])

                # --- softmax / argmax over s in [1, S) ---
                Lq = small.tile([128, S - 1], F32, tag="Lq")
                nc.vector.tensor_scalar_add(Lq[:], att[:, 1:S], 10000.0)
                m = small.tile([128, 1], F32, tag="m")
                nc.vector.reduce_max(m[:], Lq[:], axis=AX.X)
                negm = small.tile([128, 1], F32, tag="negm")
                nc.vector.tensor_scalar_mul(negm[:], m[:], -1.0)
                escr = small.tile([128, S - 1], F32, tag="escr")
                nc.scalar.activation(escr[:], Lq[:], AF.Exp, bias=negm[:],
                                     accum_out=Zbuf[cb][:, t:t + 1])
                eq = small.tile([128, S - 1], F32, tag="eq")
                nc.vector.tensor_scalar(eq[:], Lq[:], m[:], None, op0=ALU.is_equal)
                sel = small.tile([128, S - 1], F32, tag="sel")
                nc.vector.tensor_mul(sel[:], eq[:], revio[:])
                r = small.tile([128, 1], F32, tag="r")
                nc.vector.reduce_max(r[:], sel[:], axis=AX.X)
                ptrf = small.tile([128, 1], F32, tag="ptrf")
                nc.vector.tensor_scalar(ptrf[:], r[:], -1.0, float(S - 1),
                                        op0=ALU.mult, op1=ALU.add)
                nc.vector.tensor_copy(Pbuf[cb][:, t:t + 1], ptrf[:])

                # --- gather next dec ---
                if t < n_steps - 1:
                    gidxf = small.tile([128, 1], F32, tag="gidxf")
                    nc.vector.tensor_add(gidxf[:], ptrf[:], biota[cb][:])
                    gidx = small.tile([128, 1], I32, tag="gidx")
                    nc.vector.tensor_copy(gidx[:], gidxf[:])
                    dg = small.tile([128, 128], F32, tag="decg")
                    nc.gpsimd.indirect_dma_start(
                        out=dg[:], out_offset=None, in_=sh_table[:],
                        in_offset=bass.IndirectOffsetOnAxis(ap=gidx[:, :1], axis=0))
                    d_new = state.tile([128, 128], F32, tag=f"dec{cb}")
                    transpose128(dg[:], d_new[:])
                    dec_cb[cb] = d_new

        # ---------------- Phase 3: outputs ----------------
        for cb in range(NCB):
            rec = small.tile([128, n_steps], F32, tag="rec")
            nc.vector.reciprocal(rec[:], Zbuf[cb][:, :n_steps])
            lg = small.tile([128, n_steps], F32, tag="lg")
            nc.scalar.activation(lg[:], rec[:], AF.Ln)
            nc.sync.dma_start(outs["out_logp"][cb * 128:(cb + 1) * 128, :n_steps], lg[:])
            nc.sync.dma_start(outs["out_idx"][cb * 128:(cb + 1) * 128, :n_steps],
                              Pbuf[cb][:, :n_steps])
        mk = small.tile([1, 1], F32, tag="mk")
        nc.sync.dma_start(mk[:], ins["mark"][:].unsqueeze(0))
        nc.sync.dma_start(outs["out_mark"][:].unsqueeze(0), mk[:])


_CACHED = {}


def build_program(n_steps=S):
    key = n_steps
    if key in _CACHED:
        return _CACHED[key]
    nc = bacc.Bacc("TRN2", target_bir_lowering=False, debug=False,
                   num_devices=NCORES)
    ins = {}
    ins["static"] = nc.dram_tensor("static", [BC, 2, S], F32, kind="ExternalInput").ap()
    ins["dynamic"] = nc.dram_tensor("dynamic", [BC, 2, S], F32, kind="ExternalInput").ap()
    ins["mark"] = nc.dram_tensor("mark", [1], F32, kind="ExternalInput").ap()
    for nm, shp in WNAMES:
        ins[nm] = nc.dram_tensor(nm, list(shp), F32, kind="ExternalInput").ap()
    outs = {
        "out_idx": nc.dram_tensor("out_idx", [BC, S], I32, kind="ExternalOutput").ap(),
        "out_logp": nc.dram_tensor("out_logp", [BC, S], F32, kind="ExternalOutput").ap(),
        "out_mark": nc.dram_tensor("out_mark", [1], F32, kind="ExternalOutput").ap(),
    }
    with tile.TileContext(nc) as tc:
        build_body(tc, ins, outs, n_steps=n_steps)
    nc.compile()
    _CACHED[key] = nc
    return nc


LAST_RUN_INFO = {}


def kernel(**inputs):
    inp = {k: np.ascontiguousarray(np.asarray(v)) for k, v in inputs.items()}
    nc = build_program(S)
    in_maps = []
    for c in range(NCORES):
        m = {
            "static": inp["static"][c * BC:(c + 1) * BC].astype(np.float32, copy=False),
            "dynamic": inp["dynamic"][c * BC:(c + 1) * BC].astype(np.float32, copy=False),
            "mark": inp["mark"].astype(np.float32, copy=False),
        }
        for nm, _ in WNAMES:
            m[nm] = inp[nm].astype(np.float32, copy=False)
        in_maps.append(m)
    t0 = time.time()
    trace = bool(int(os.environ.get("KERNEL_TRACE", "0")))
    res = run_bass_kernel_spmd(nc, in_maps, list(range(NCORES)), trace=trace)
    LAST_RUN_INFO["wall_s"] = time.time() - t0
    LAST_RUN_INFO["exec_time_ns"] = getattr(res, "exec_time_ns", None)
    LAST_RUN_INFO["profile_json"] = getattr(res, "profile_json", None)
    rs = res.results
    tour_idx = np.concatenate([rs[c]["out_idx"] for c in range(NCORES)], axis=0)
    tour_logp = np.concatenate([rs[c]["out_logp"] for c in range(NCORES)], axis=0)
    mark = rs[0]["out_mark"]
    return tour_idx.astype(np.int32), tour_logp.astype(np.float32), mark.astype(np.float32)


# revision 9
# speedup vs baseline: 1.0011x; 1.0011x over previous
"""DRL4TSP greedy pointer-network decode on 8 Trainium2 NeuronCores.

Pure batch data parallelism: 2048 rows -> 8 cores x 256 rows. All weights
replicated. Per core, the S=128 sequential decode steps run on-chip:

  layouts (per core, BC=256 rows split into 2 chunks of 128 "cb"):
    base      SBUF [h=128, b=256, s=128] fp32   (loop-invariant attention term)
    sh_table  DRAM [b*S=32768, h=128]    fp32   (static_hidden rows for gather)
    state     SBUF [h=128, b=128] per cb        (h_t, c_t, dec_t, q_t)

  per step: LSTM gates (PE fp32) -> sigmoid-via-tanh / tanh (ACT LUT) ->
  q proj (PE) -> u = base + q broadcast (DVE) -> tanh(u) (ACT) ->
  attns = w . tanh(u) as per-b self-loading fp32 matmuls (PE, psum [s,b]) ->
  transpose -> softmax over s>=1 with the reference's +10000 fp32
  quantization reproduced -> argmax with first-index tie-break (DVE) ->
  indirect-DMA gather of the next decoder input.

Numerics: ACT's tanh/exp/log LUTs are bit-identical to XLA-CPU's, fp32 PE
matmuls are fp32-faithful, so attns track the CPU-jax reference to ~1e-7,
which the +10000 quantization (grid ~1e-3) absorbs.
"""
import os
import time
from contextlib import ExitStack

import numpy as np

import concourse.bass as bass
import concourse.tile as tile
from concourse import bacc, mybir
from concourse.bass_utils import run_bass_kernel_spmd
from concourse.masks import make_identity

dt = mybir.dt
F32 = dt.float32
I32 = dt.int32
AF = mybir.ActivationFunctionType
ALU = mybir.AluOpType
AX = mybir.AxisListType

NCORES = 8
B = 2048
BC = B // NCORES          # 256 rows per core
S = 128                   # sequence length == number of decode steps
H = 128                   # hidden
NCB = BC // 128           # 2 b-chunks of 128 per core
CHUNK_B = 16              # b's per u/tanh chunk (chunk = [128, 16*128] cols)

WNAMES = [
    ("W_s", (H, 2)), ("b_s", (H,)), ("W_ld", (H, 2)), ("b_ld", (H,)),
    ("W_d", (H, 2)), ("b_d", (H,)),
    ("W_ih", (4 * H, H)), ("b_ih", (4 * H,)), ("W_hh", (4 * H, H)), ("b_hh", (4 * H,)),
    ("W_pd", (H, H)), ("b_pd", (H,)), ("W_pld", (H, H)), ("b_pld", (H,)),
    ("W_pq", (H, H)), ("b_pq", (H,)), ("W_pr", (H, H)), ("b_pr", (H,)),
    ("attn_W", (1, H)),
]


def build_body(tc, ins, outs, n_steps=S):
    """Emit the whole per-core program. ins/outs: dicts name -> DRAM AP."""
    nc = tc.nc
    with ExitStack() as ctx:
        const = ctx.enter_context(tc.tile_pool(name="const", bufs=1))
        basep = ctx.enter_context(tc.tile_pool(name="basep", bufs=1))
        accp = ctx.enter_context(tc.tile_pool(name="accp", bufs=1))
        dramp = ctx.enter_context(tc.tile_pool(name="dramp", bufs=1, space="DRAM"))
        state = ctx.enter_context(tc.tile_pool(name="state", bufs=2))
        work = ctx.enter_context(tc.tile_pool(name="work", bufs=2))
        small = ctx.enter_context(tc.tile_pool(name="small", bufs=2))
        psg = ctx.enter_context(tc.tile_pool(name="psg", bufs=2, space="PSUM"))
        psq = ctx.enter_context(tc.tile_pool(name="psq", bufs=2, space="PSUM"))
        psattn = ctx.enter_context(tc.tile_pool(name="psattn", bufs=2, space="PSUM"))
        pst = ctx.enter_context(tc.tile_pool(name="pst", bufs=2, space="PSUM"))

        ident = const.tile([128, 128], F32, tag="ident")
        make_identity(nc, ident[:])

        def transpose128(src_ap, dst_ap, engine=None):
            p = pst.tile([128, 128], F32, tag="pst")
            nc.tensor.transpose(p[:], src_ap, ident[:])
            if engine is nc.scalar:
                nc.scalar.copy(dst_ap, p[:])
            else:
                nc.vector.tensor_copy(dst_ap, p[:])

        # ---------------- Phase 0: weights prep ----------------
        def load_t(name, shape, tag):
            t = const.tile(list(shape), F32, tag=tag)
            nc.sync.dma_start(t[:], ins[name][:])
            return t

        # [512,128] -> 4 transposed blocks [128(k),128(gate_h)]
        def blocks_T(name, tag):
            blks = []
            for g in range(4):
                raw = small.tile([128, 128], F32, tag="rawblk")
                nc.sync.dma_start(raw[:], ins[name][g * 128:(g + 1) * 128, :])
                tt = const.tile([128, 128], F32, tag=f"{tag}{g}")
                transpose128(raw[:], tt[:])
                blks.append(tt)
            return blks

        W_ihT = blocks_T("W_ih", "wihT")
        W_hhT = blocks_T("W_hh", "whhT")

        def square_T(name, tag):
            raw = small.tile([128, 128], F32, tag="rawblk")
            nc.sync.dma_start(raw[:], ins[name][:])
            tt = const.tile([128, 128], F32, tag=tag)
            transpose128(raw[:], tt[:])
            return tt

        W_pqT = square_T("W_pq", "wpqT")
        W_prT = square_T("W_pr", "wprT")
        W_pldT = square_T("W_pld", "wpldT")
        W_pdT = square_T("W_pd", "wpdT")

        # bias columns [128,1] via partition-scatter DMA
        def col_of(vec_ap, tag, n=128):
            c = const.tile([n, 1], F32, tag=tag)
            nc.sync.dma_start(c[:], vec_ap.unsqueeze(1))
            return c

        b_s_c = col_of(ins["b_s"][:], "bsc")
        b_ld_c = col_of(ins["b_ld"][:], "bldc")
        b_d_c = col_of(ins["b_d"][:], "bdc")
        b_pq_c = col_of(ins["b_pq"][:], "bpqc")
        wcol = col_of(ins["attn_W"][0, :], "wcol")

        # gate biases: bg = b_ih + b_hh as [128, 4]; bgh = 0.5*bg
        bihc = const.tile([128, 4], F32, tag="bihc")
        bhhc = const.tile([128, 4], F32, tag="bhhc")
        nc.sync.dma_start(bihc[:], ins["b_ih"][:].rearrange("(g p) -> p g", p=128))
        nc.sync.dma_start(bhhc[:], ins["b_hh"][:].rearrange("(g p) -> p g", p=128))
        bg = const.tile([128, 4], F32, tag="bg")
        nc.vector.tensor_add(bg[:], bihc[:], bhhc[:])
        bgh = const.tile([128, 4], F32, tag="bgh")
        nc.vector.tensor_scalar_mul(bgh[:], bg[:], 0.5)

        # W_s [128,2]; effective 1-col sums of W_ld / W_d (einsum broadcast)
        W_s_sb = load_t("W_s", (128, 2), "wssb")
        W_ld_sb = load_t("W_ld", (128, 2), "wldsb")
        W_d_sb = load_t("W_d", (128, 2), "wdsb")
        wldsum = const.tile([128, 1], F32, tag="wldsum")
        nc.vector.tensor_reduce(wldsum[:], W_ld_sb[:], axis=AX.X, op=ALU.add)
        wdsum = const.tile([128, 1], F32, tag="wdsum")
        nc.vector.tensor_reduce(wdsum[:], W_d_sb[:], axis=AX.X, op=ALU.add)

        # feature row order: 0=ONES, 1=X0, 2=X1, 3=LD, 4=DD.
        # Engine ops must start at partition 0, so rows are staged at
        # partition 0 and DMA'd into place.
        lhsT_base = const.tile([5, 128], F32, tag="lhsTbase")
        lhsT_sh = const.tile([5, 128], F32, tag="lhsTsh")

        def stage_to(dst_row_ap, psum_ap, nrows):
            stg = small.tile([nrows, 128], F32, tag="stgrow", name="stgrow")
            nc.vector.tensor_copy(stg[:], psum_ap)
            nc.sync.dma_start(dst_row_ap, stg[:])

        # rows 1:3 of lhsT_base = A_sT = W_s^T @ W_pr^T
        p = pst.tile([128, 128], F32, tag="pst")
        nc.tensor.matmul(p[0:2, :], W_s_sb[:], W_prT[:], start=True, stop=True)
        stage_to(lhsT_base[1:3, :], p[0:2, :], 2)
        # row 3 = A_ldT (effective 1-col), row 4 = A_dT
        p = pst.tile([128, 128], F32, tag="pst")
        nc.tensor.matmul(p[0:1, :], wldsum[:], W_pldT[:], start=True, stop=True)
        stage_to(lhsT_base[3:4, :], p[0:1, :], 1)
        p = pst.tile([128, 128], F32, tag="pst")
        nc.tensor.matmul(p[0:1, :], wdsum[:], W_pdT[:], start=True, stop=True)
        stage_to(lhsT_base[4:5, :], p[0:1, :], 1)
        # row 0 = btot = W_pr@b_s + W_pld@b_ld + W_pd@b_d + b_pr + b_pld + b_pd
        p = pst.tile([128, 128], F32, tag="pst")
        nc.tensor.matmul(p[0:1, :], b_s_c[:], W_prT[:], start=True, stop=False)
        nc.tensor.matmul(p[0:1, :], b_ld_c[:], W_pldT[:], start=False, stop=False)
        nc.tensor.matmul(p[0:1, :], b_d_c[:], W_pdT[:], start=False, stop=True)
        btot = small.tile([1, 128], F32, tag="btot")
        nc.vector.tensor_copy(btot[:], p[0:1, :])
        for nm in ("b_pr", "b_pld", "b_pd"):
            brow = small.tile([1, 128], F32, tag="brow")
            nc.sync.dma_start(brow[:], ins[nm][:].unsqueeze(0))
            nc.vector.tensor_add(btot[:], btot[:], brow[:])
        nc.sync.dma_start(lhsT_base[0:1, :], btot[:])

        # lhsT_sh rows: 0 = b_s, 1:3 = W_sT, 3:5 = zeros
        bsrow = small.tile([1, 128], F32, tag="brow")
        nc.sync.dma_start(bsrow[:], ins["b_s"][:].unsqueeze(0))
        nc.sync.dma_start(lhsT_sh[0:1, :], bsrow[:])
        p = pst.tile([128, 128], F32, tag="pst")
        nc.tensor.matmul(p[0:2, :], W_s_sb[:], ident[:], start=True, stop=True)
        stage_to(lhsT_sh[1:3, :], p[0:2, :], 2)
        zrow = small.tile([1, 128], F32, tag="zrow")
        nc.vector.memset(zrow[:], 0.0)
        nc.sync.dma_start(lhsT_sh[3:4, :], zrow[:])
        nc.sync.dma_start(lhsT_sh[4:5, :], zrow[:])

        # iotas
        revio_i = const.tile([128, S - 1], I32, tag="revioi")
        nc.gpsimd.iota(revio_i[:], pattern=[[-1, S - 1]], base=S - 2, channel_multiplier=0)
        revio = const.tile([128, S - 1], F32, tag="revio")
        nc.vector.tensor_copy(revio[:], revio_i[:])
        biota = []
        for cb in range(NCB):
            bi = const.tile([128, 1], I32, tag=f"biotai{cb}")
            nc.gpsimd.iota(bi[:], pattern=[[0, 1]], base=cb * 128 * S, channel_multiplier=S)
            bf = const.tile([128, 1], F32, tag=f"biotaf{cb}")
            nc.vector.tensor_copy(bf[:], bi[:])
            biota.append(bf)

        # ---------------- Phase 1: base + sh_table ----------------
        base_t = basep.tile([128, BC, S], F32, tag="base")
        sh_table = dramp.tile([BC * S, H], F32, tag="shtab")

        # LD = load - demand, staged via DRAM so phase-1 chunks can slice rows
        LDfull = const.tile([128, NCB, S], F32, tag="ldfull")
        ld_dram = dramp.tile([BC, S], F32, tag="lddram", name="lddram")
        for cb in range(NCB):
            dyn = work.tile([128, 2, S], F32, tag="u")
            nc.sync.dma_start(dyn[:], ins["dynamic"][cb * 128:(cb + 1) * 128, :, :])
            nc.vector.tensor_sub(LDfull[:, cb, :], dyn[:, 0, :], dyn[:, 1, :])
            nc.sync.dma_start(ld_dram[cb * 128:(cb + 1) * 128, :], LDfull[:, cb, :])

        st_r = ins["static"].rearrange("b c s -> c b s")
        dy_r = ins["dynamic"].rearrange("b c s -> c b s")
        NB_CH = 16                      # b's per phase-1 chunk (2048 cols)
        for ch in range(BC // NB_CH):
            b0 = ch * NB_CH
            feat = work.tile([5, NB_CH, S], F32, tag="u")
            nc.vector.memset(feat[0:1, :, :], 1.0)
            nc.sync.dma_start(feat[1:2, :, :], st_r[0:1, b0:b0 + NB_CH, :])
            nc.sync.dma_start(feat[2:3, :, :], st_r[1:2, b0:b0 + NB_CH, :])
            nc.sync.dma_start(feat[3:4, :, :], ld_dram[b0:b0 + NB_CH, :].unsqueeze(0))
            nc.sync.dma_start(feat[4:5, :, :], dy_r[1:2, b0:b0 + NB_CH, :])
            featf = feat[:].rearrange("k b s -> k (b s)")
            for sub in range(NB_CH * S // 512):
                cols = featf[:, sub * 512:(sub + 1) * 512]
                pb = psq.tile([128, 512], F32, tag="q")
                nc.tensor.matmul(pb[:], lhsT_base[:], cols, start=True, stop=True)
                nc.vector.tensor_copy(
                    base_t[:].rearrange("h b s -> h (b s)")[:, ch * NB_CH * S + sub * 512:][:, :512],
                    pb[:])
                n0 = ch * NB_CH * S + sub * 512
                for blk in range(4):
                    # directly transposed: out[n,h] = feat[:,ncols].T @ W'
                    psh = psg.tile([128, 128], F32, tag="gates")
                    nc.tensor.matmul(psh[:], cols[:, blk * 128:(blk + 1) * 128],
                                     lhsT_sh[:], start=True, stop=True)
                    stT = small.tile([128, 128], F32, tag="stT")
                    nc.scalar.copy(stT[:], psh[:])
                    nc.sync.dma_start(
                        sh_table[n0 + blk * 128:n0 + (blk + 1) * 128, :], stT[:])

        # ---------------- Phase 2: decode loop ----------------
        # initial state
        h_cb, c_cb, dec_cb = [], [], []
        for cb in range(NCB):
            h0 = state.tile([128, 128], F32, tag=f"h{cb}")
            nc.vector.memset(h0[:], 0.0)
            c0 = state.tile([128, 128], F32, tag=f"c{cb}")
            nc.vector.memset(c0[:], 0.0)
            dg = small.tile([128, 128], F32, tag="decg")
            nc.sync.dma_start(
                dg[:],
                sh_table[:].rearrange("(b s) h -> b s h", s=S)[cb * 128:(cb + 1) * 128, 0, :])
            d0 = state.tile([128, 128], F32, tag=f"dec{cb}")
            transpose128(dg[:], d0[:])
            h_cb.append(h0); c_cb.append(c0); dec_cb.append(d0)

        Zbuf, Pbuf = [], []
        for cb in range(NCB):
            Zbuf.append(accp.tile([128, S], F32, tag=f"zbuf{cb}", name=f"zbuf{cb}"))
            Pbuf.append(accp.tile([128, S], I32, tag=f"pbuf{cb}", name=f"pbuf{cb}"))

        NCH = 128 // CHUNK_B            # u/tanh chunks per cb
        for t in range(n_steps):
            for cb in range(NCB):
                # --- LSTM ---
                pg = psg.tile([128, 4, 128], F32, tag="gates")
                for g in range(4):
                    nc.tensor.matmul(pg[:, g, :], W_ihT[g][:], dec_cb[cb][:],
                                     start=True, stop=False)
                    nc.tensor.matmul(pg[:, g, :], W_hhT[g][:], h_cb[cb][:],
                                     start=False, stop=True)
                gs = []
                for g in (0, 1, 3):     # i, f, o: sigmoid via tanh
                    th = small.tile([128, 128], F32, tag=f"gth{g}")
                    nc.scalar.activation(th[:], pg[:, g, :], AF.Tanh,
                                         bias=bgh[:, g:g + 1], scale=0.5)
                    sg = small.tile([128, 128], F32, tag=f"gsg{g}")
                    nc.vector.tensor_scalar(sg[:], th[:], 0.5, 0.5,
                                            op0=ALU.mult, op1=ALU.add)
                    gs.append(sg)
                i_s, f_s, o_s = gs
                g_t = small.tile([128, 128], F32, tag="gcell")
                nc.scalar.activation(g_t[:], pg[:, 2, :], AF.Tanh,
                                     bias=bg[:, 2:3], scale=1.0)
                t1 = small.tile([128, 128], F32, tag="t1")
                nc.vector.tensor_mul(t1[:], f_s[:], c_cb[cb][:])
                t2 = small.tile([128, 128], F32, tag="t2")
                nc.vector.tensor_mul(t2[:], i_s[:], g_t[:])
                c_new = state.tile([128, 128], F32, tag=f"c{cb}")
                nc.vector.tensor_add(c_new[:], t1[:], t2[:])
                ct = small.tile([128, 128], F32, tag="ct")
                nc.scalar.activation(ct[:], c_new[:], AF.Tanh)
                h_new = state.tile([128, 128], F32, tag=f"h{cb}")
                nc.vector.tensor_mul(h_new[:], o_s[:], ct[:])
                c_cb[cb] = c_new; h_cb[cb] = h_new

                # --- q ---
                pq = psq.tile([128, 128], F32, tag="q")
                nc.tensor.matmul(pq[:], W_pqT[:], h_new[:], start=True, stop=True)
                qS = state.tile([128, 128], F32, tag=f"q{cb}")
                nc.scalar.activation(qS[:], pq[:], AF.Identity, bias=b_pq_c[:])

                # --- u = base + q, tanh, attn matmuls ---
                pa = psattn.tile([128, 128], F32, tag="attn")
                for ch in range(NCH):
                    bl = cb * 128 + ch * CHUNK_B     # global b of chunk start
                    u = work.tile([128, CHUNK_B, S], F32, tag="u")
                    nc.vector.tensor_tensor(
                        u[:], base_t[:, bl:bl + CHUNK_B, :],
                        qS[:, ch * CHUNK_B:(ch + 1) * CHUNK_B].unsqueeze(2)
                          .broadcast_to([128, CHUNK_B, S]),
                        op=ALU.add)
                    tt = work.tile([128, CHUNK_B, S], F32, tag="tt")
                    nc.scalar.activation(tt[:], u[:], AF.Tanh)
                    for j in range(CHUNK_B):
                        jb = ch * CHUNK_B + j
                        nc.tensor.matmul(pa[:, jb:jb + 1], tt[:, j, :], wcol[:],
                                         start=True, stop=True)

                # --- evac + transpose to [b, s] ---
                aT = small.tile([128, 128], F32, tag="aT")
                nc.scalar.copy(aT[:], pa[:])
                att = small.tile([128, 128], F32, tag="att")
                transpose128(aT[:], att[:])

                # --- softmax / argmax over s in [1, S) ---
                Lq = small.tile([128, S - 1], F32, tag="Lq")
                nc.vector.tensor_scalar_add(Lq[:], att[:, 1:S], 10000.0)
                m = small.tile([128, 1], F32, tag="m")
                nc.vector.reduce_max(m[:], Lq[:], axis=AX.X)
                negm = small.tile([128, 1], F32, tag="negm")
                nc.vector.tensor_scalar_mul(negm[:], m[:], -1.0)
                escr = small.tile([128, S - 1], F32, tag="escr")
                nc.scalar.activation(escr[:], Lq[:], AF.Exp, bias=negm[:],
                                     accum_out=Zbuf[cb][:, t:t + 1])
                eq = small.tile([128, S - 1], F32, tag="eq")
                nc.vector.tensor_scalar(eq[:], Lq[:], m[:], None, op0=ALU.is_equal)
                sel = small.tile([128, S - 1], F32, tag="sel")
                nc.vector.tensor_mul(sel[:], eq[:], revio[:])
                r = small.tile([128, 1], F32, tag="r")
                nc.vector.reduce_max(r[:], sel[:], axis=AX.X)
                ptrf = small.tile([128, 1], F32, tag="ptrf")
                nc.vector.tensor_scalar(ptrf[:], r[:], -1.0, float(S - 1),
                                        op0=ALU.mult, op1=ALU.add)
                nc.vector.tensor_copy(Pbuf[cb][:, t:t + 1], ptrf[:])

                # --- gather next dec ---
                if t < n_steps - 1:
                    gidxf = small.tile([128, 1], F32, tag="gidxf")
                    nc.vector.tensor_add(gidxf[:], ptrf[:], biota[cb][:])
                    gidx = small.tile([128, 1], I32, tag="gidx")
                    nc.vector.tensor_copy(gidx[:], gidxf[:])
                    dg = small.tile([128, 128], F32, tag="decg")
                    nc.gpsimd.indirect_dma_start(
                        out=dg[:], out_offset=None, in_=sh_table[:],
                        in_offset=bass.IndirectOffsetOnAxis(ap=gidx[:, :1], axis=0))
                    d_new = state.tile([128, 128], F32, tag=f"dec{cb}")
                    transpose128(dg[:], d_new[:])
                    dec_cb[cb] = d_new

        # ---------------- Phase 3: outputs ----------------
        for cb in range(NCB):
            rec = small.tile([128, n_steps], F32, tag="rec")
            nc.vector.reciprocal(rec[:], Zbuf[cb][:, :n_steps])
            lg = small.tile([128, n_steps], F32, tag="lg")
            nc.scalar.activation(lg[:], rec[:], AF.Ln)
            nc.sync.dma_start(outs["out_logp"][cb * 128:(cb + 1) * 128, :n_steps], lg[:])
            nc.sync.dma_start(outs["out_idx"][cb * 128:(cb + 1) * 128, :n_steps],
                              Pbuf[cb][:, :n_steps])
        mk = small.tile([1, 1], F32, tag="mk")
        nc.sync.dma_start(mk[:], ins["mark"][:].unsqueeze(0))
        nc.sync.dma_start(outs["out_mark"][:].unsqueeze(0), mk[:])


_CACHED = {}


def build_program(n_steps=S):
    key = n_steps
    if key in _CACHED:
        return _CACHED[key]
    nc = bacc.Bacc("TRN2", target_bir_lowering=False, debug=False,
                   num_devices=NCORES)
    ins = {}
    ins["static"] = nc.dram_tensor("static", [BC, 2, S], F32, kind="ExternalInput").ap()
    ins["dynamic"] = nc.dram_tensor("dynamic", [BC, 2, S], F32, kind="ExternalInput").ap()
    ins["mark"] = nc.dram_tensor("mark", [1], F32, kind="ExternalInput").ap()
    for nm, shp in WNAMES:
        ins[nm] = nc.dram_tensor(nm, list(shp), F32, kind="ExternalInput").ap()
    outs = {
        "out_idx": nc.dram_tensor("out_idx", [BC, S], I32, kind="ExternalOutput").ap(),
        "out_logp": nc.dram_tensor("out_logp", [BC, S], F32, kind="ExternalOutput").ap(),
        "out_mark": nc.dram_tensor("out_mark", [1], F32, kind="ExternalOutput").ap(),
    }
    with tile.TileContext(nc) as tc:
        build_body(tc, ins, outs, n_steps=n_steps)
    nc.compile()
    _CACHED[key] = nc
    return nc


LAST_RUN_INFO = {}


def kernel(**inputs):
    inp = {k: np.ascontiguousarray(np.asarray(v)) for k, v in inputs.items()}
    nc = build_program(S)
    in_maps = []
    for c in range(NCORES):
        m = {
            "static": inp["static"][c * BC:(c + 1) * BC].astype(np.float32, copy=False),
            "dynamic": inp["dynamic"][c * BC:(c + 1) * BC].astype(np.float32, copy=False),
            "mark": inp["mark"].astype(np.float32, copy=False),
        }
        for nm, _ in WNAMES:
            m[nm] = inp[nm].astype(np.float32, copy=False)
        in_maps.append(m)
    t0 = time.time()
    trace = bool(int(os.environ.get("KERNEL_TRACE", "0")))
    res = run_bass_kernel_spmd(nc, in_maps, list(range(NCORES)), trace=trace)
    LAST_RUN_INFO["wall_s"] = time.time() - t0
    LAST_RUN_INFO["exec_time_ns"] = getattr(res, "exec_time_ns", None)
    LAST_RUN_INFO["profile_json"] = getattr(res, "profile_json", None)
    rs = res.results
    tour_idx = np.concatenate([rs[c]["out_idx"] for c in range(NCORES)], axis=0)
    tour_logp = np.concatenate([rs[c]["out_logp"] for c in range(NCORES)], axis=0)
    mark = rs[0]["out_mark"]
    return tour_idx.astype(np.int32), tour_logp.astype(np.float32), mark.astype(np.float32)
